# revision 41
# baseline (speedup 1.0000x reference)
# Causal multi-head self-attention (fused QKV) on 8 Trainium2 NeuronCores.
#
# Problem shapes (hardcoded): x [2, 2048, 1024], W_qkv [3072, 1024],
# b_qkv [3072]; H=16 heads, head_dim 64.
#
# Sharding: core c handles batch b = c//4 and head group hg = c%4 (4 heads).
# Each core:
#   - projects its 512 Q/K rows of W_qkv transposed   -> qkT [512, 2048]
#     (rows = (head, head_dim), cols = seq; i.e. Q^T / K^T per head)
#   - projects its 256 V rows non-transposed          -> v   [2048, 256]
#   - attention per head: S^T = K^T.T @ Q^T tiles (contraction over head
#     dim on partitions), exp on ACT with causal block skipping, then
#     out' = [V | 1]^T-style matmul so the softmax denominator falls out
#     of the same accumulation as the numerator (extra stationary column
#     of ones).  out'[65, seq] per head is DMA'd out unnormalized; the
#     host divides by row 64 and transposes during the gather.
#
# Matmuls run as float32r (full PE rate, fp32 operands) by default;
# set CMSA_CMODE=bf16 or f32 to switch.

import os

import numpy as np

BS, SEQ, D, H, HD = 2, 2048, 1024, 16, 64
NCORES = 8
GROUPS = NCORES // BS          # 4 head-groups per batch
HPG = H // GROUPS              # 4 heads per group/core
NQK = 2 * HPG * HD             # 512 rows of Q+K per core
NV = HPG * HD                  # 256 rows of V per core
P = 128
KT = SEQ // P                  # 16 k-tiles per head
QCH = 1024                     # q-chunk width (2 psum banks)
NJ = SEQ // QCH                # 2 q-chunks
DT_TILES = D // P              # 8 contraction tiles for the projection
HDP = HD + 2                   # per-head stride in v4: V | ones | zero-pad
                               # (even stationary width for the fp32r ISA rules)

CMODE = os.environ.get("CMSA_CMODE", "f32r")

_CACHE = {}


def _build_program():
    import concourse.mybir as mybir
    import concourse.tile as tile
    from concourse import bacc

    fp32 = mybir.dt.float32
    if CMODE == "bf16":
        cdt = mybir.dt.bfloat16
    elif CMODE == "f32":
        cdt = mybir.dt.float32
    else:
        # float32r: fp32-width operands at full PE rate; every producer of a
        # matmul operand must itself emit float32r, so the whole tile chain
        # (and the DRAM inputs they're DMA'd from) is typed float32r.
        cdt = mybir.dt.float32r

    def mm(ap):
        return ap

    nc = bacc.Bacc(
        "TRN2",
        target_bir_lowering=False,
        debug=False,
        enable_asserts=False,
        num_devices=NCORES,
    )

    # ---- DRAM I/O ----
    xT_d = nc.dram_tensor("xT", [D, SEQ], cdt, kind="ExternalInput").ap()
    wqk_d = nc.dram_tensor("wqk", [D, NQK], cdt, kind="ExternalInput").ap()
    wv_d = nc.dram_tensor("wv", [D, NV], cdt, kind="ExternalInput").ap()
    bqk_d = nc.dram_tensor("bqk", [NQK, 1], fp32, kind="ExternalInput").ap()
    bv_d = nc.dram_tensor("bv", [P, NV], fp32, kind="ExternalInput").ap()
    tri_d = nc.dram_tensor("tri", [P, P], cdt, kind="ExternalInput").ap()

    kT_d = nc.dram_tensor("kT", [NQK // 2, SEQ], cdt, kind="ExternalOutput").ap()
    v_d = nc.dram_tensor("v", [SEQ, NV], cdt, kind="ExternalOutput").ap()
    outT_d = nc.dram_tensor("outT", [HPG * (HD + 1), SEQ], fp32, kind="ExternalOutput").ap()

    with tile.TileContext(nc) as tc:
        from contextlib import ExitStack

        with ExitStack() as ctx:
            consts = ctx.enter_context(tc.tile_pool(name="consts", bufs=1))
            xt_pool = ctx.enter_context(tc.tile_pool(name="xt", bufs=1))
            w_pool = ctx.enter_context(tc.tile_pool(name="w", bufs=1))
            qk_pool = ctx.enter_context(tc.tile_pool(name="qk", bufs=1))
            v4_pool = ctx.enter_context(tc.tile_pool(name="v4", bufs=1))
            pt_pool = ctx.enter_context(tc.tile_pool(name="pt", bufs=4))
            ot_pool = ctx.enter_context(tc.tile_pool(name="ot", bufs=2))
            # PSUM budget (8 banks): proj/scores 2x[128,1024] (4) + PV
            # accumulators 2x[66,1024] (4)
            psA = ctx.enter_context(tc.tile_pool(name="psA", bufs=2, space="PSUM"))
            psB = ctx.enter_context(tc.tile_pool(name="psB", bufs=2, space="PSUM"))

            # ---- load inputs ----
            # (wqk_d, xt_d) pairs first so the projection's d-accumulation can
            # start as soon as the first pair lands rather than after the full
            # 11MB input load.
            tri = consts.tile([P, P], cdt, name="tri", tag="tri")
            nc.sync.dma_start(tri[:], tri_d[:, :])
            bqk = consts.tile([P, NQK // P], fp32, name="bqk", tag="bqk")
            for nt in range(NQK // P):
                nc.sync.dma_start(bqk[:, nt:nt + 1], bqk_d[nt * P:(nt + 1) * P, :])
            bv = consts.tile([P, NV], fp32, name="bv", tag="bv")
            nc.sync.dma_start(bv[:], bv_d[:, :])
            xt = [xt_pool.tile([P, SEQ], cdt, name=f"xt{d}", tag=f"xt{d}") for d in range(DT_TILES)]
            wqk = [w_pool.tile([P, NQK], cdt, name=f"wqk{d}", tag=f"wqk{d}") for d in range(DT_TILES)]
            wv = [w_pool.tile([P, NV], cdt, name=f"wv{d}", tag=f"wv{d}") for d in range(DT_TILES)]
            # wqk tiles whole (contiguous rows, only 2MB total), xt in
            # column quarters delivered in consumption order so the first
            # projection chunk starts after ~4MB instead of the full 10MB.
            for d in range(DT_TILES):
                nc.sync.dma_start(wqk[d][:], wqk_d[d * P:(d + 1) * P, :])
            for blk in range(4):
                for d in range(DT_TILES):
                    nc.sync.dma_start(
                        xt[d][:, blk * 512:(blk + 1) * 512],
                        xT_d[d * P:(d + 1) * P, blk * 512:(blk + 1) * 512],
                    )
            for d in range(DT_TILES):
                nc.sync.dma_start(wv[d][:], wv_d[d * P:(d + 1) * P, :])
            # fp32 constant tiles (gpsimd memset can't write float32r; DVE
            # copies from these do the fp32 -> fp32r rounding instead)
            oz = consts.tile([P, 2 * HPG], fp32, name="oz", tag="oz")
            ozs = oz[:].rearrange("p (h e) -> p h e", h=HPG)
            nc.gpsimd.memset(ozs[:, :, 0:1], 1.0)
            nc.gpsimd.memset(ozs[:, :, 1:2], 0.0)
            zer = consts.tile([P, SEQ], fp32, name="zer", tag="zer")
            nc.gpsimd.memset(zer[:], 0.0)

            # ---- projection: qkT[n, s] = sum_d wqk[d, n] * xT[d, s] + bqk[n]
            # Q^T stays packed two-heads-per-tile (it is the 128-row moving
            # operand of the scores matmul).  K^T goes into per-head tiles
            # zero-padded to 128 partitions, so the scores stationary is a
            # full (128,128) tile — same PE config as every other matmul
            # (the pad rows hit the odd head's Q rows and contribute 0).
            # K^T of head h sits at rows 64*(h%2)..+64 (aligned with that
            # head's Q^T rows in the packed q tile); the other 64 rows are 0.
            q_tiles = [qk_pool.tile([P, SEQ], cdt, name=f"q{nt}", tag=f"q{nt}") for nt in range(2)]
            kp_tiles = [qk_pool.tile([P, SEQ], cdt, name=f"kp{h}", tag=f"kp{h}") for h in range(HPG)]
            for h in range(HPG):
                zrow = 64 * (1 - h % 2)
                nc.vector.tensor_copy(kp_tiles[h][zrow:zrow + 64, :], zer[0:64, :])

            def proj_qk(nt):
                for sc in range(4):
                    o = sc * 512
                    psw = psA.tile([P, QCH], fp32, name="psA", tag="psA")
                    ps = psw[:, 0:512]
                    for d in range(DT_TILES):
                        nc.tensor.matmul(
                            ps[:],
                            mm(wqk[d][:, nt * P:(nt + 1) * P]),
                            mm(xt[d][:, o:o + 512]),
                            start=(d == 0),
                            stop=(d == DT_TILES - 1),
                        )
                    if nt < 2:
                        nc.vector.tensor_scalar_add(
                            q_tiles[nt][:, o:o + 512], ps[:], bqk[:, nt:nt + 1]
                        )
                    else:
                        for hh in range(2):
                            h = (nt - 2) * 2 + hh
                            r = hh * 64
                            nc.vector.tensor_scalar_add(
                                kp_tiles[h][r:r + 64, o:o + 512],
                                ps[r:r + 64, :],
                                bqk[r:r + 64, nt:nt + 1],
                            )
                            nc.sync.dma_start(
                                kT_d[h * 64:(h + 1) * 64, o:o + 512],
                                kp_tiles[h][r:r + 64, o:o + 512],
                            )

            # ---- projection: v[s, n] = sum_d xT[d, s] * wv[d, n] + bv[n]
            # v4 tiles hold [V | 1] interleaved per head: cols h*65..h*65+63 = V,
            # col h*65+64 = ones (PV stationary with free denominator row).
            v4_tiles = []

            def proj_v(sts):
                for st in sts:
                    psw = psA.tile([P, QCH], fp32, name="psA", tag="psA")
                    ps = psw[:, 0:512]
                    for d in range(DT_TILES):
                        nc.tensor.matmul(
                            ps[:, 0:NV],
                            mm(xt[d][:, st * P:(st + 1) * P]),
                            mm(wv[d][:]),
                            start=(d == 0),
                            stop=(d == DT_TILES - 1),
                        )
                    v4 = v4_pool.tile([P, HPG * HDP], cdt, name=f"v4_{st}", tag=f"v4_{st}")
                    v4r = v4[:].rearrange("p (h e) -> p h e", h=HPG)
                    v4s = v4r[:, :, 0:HD]
                    nc.vector.tensor_copy(v4r[:, :, HD:HDP], ozs)
                    nc.vector.tensor_add(
                        v4s,
                        ps[:, 0:NV].rearrange("p (h e) -> p h e", h=HPG),
                        bv[:].rearrange("p (h e) -> p h e", h=HPG),
                    )
                    nc.sync.dma_start(
                        v_d[st * P:(st + 1) * P, :].rearrange("p (h e) -> p h e", h=HPG),
                        v4s,
                    )
                    v4_tiles.append(v4)

            # ---- attention for one head, one q-chunk ----
            def attn(h, j):
                ktile = kp_tiles[h]
                qtile = q_tiles[h // 2]
                n_k = 8 * (j + 1)
                # diagonal k-tiles first: their serial exp chains overlap the
                # full-tile matmuls instead of sitting on the kernel tail
                order = list(range(8 * j, n_k)) + list(range(0, 8 * j))
                halves = {0: [i for i in order if i - 8 * j < 4], 1: order}
                pso = psB.tile([HD + 2, QCH], fp32, name="pso", tag="pso")
                for i in order:
                    m = i - 8 * j  # >= 0 on the diagonal band
                    vs = max(P * m, 0)  # first causally-valid q col in chunk
                    pss = psA.tile([P, QCH], fp32, name="psA", tag="psA")
                    pt = pt_pool.tile([P, QCH], cdt, name="pt", tag="pt")
                    # per 512-half, only the causally valid segment [s0:s1):
                    # scores/PV stream just that range, so the masked-out
                    # left part of pt is never written NOR read.
                    segs = []
                    for half in range(2):
                        s0, s1 = max(half * 512, vs), (half + 1) * 512
                        if s0 < s1:
                            segs.append((half, s0, s1))
                    for half, s0, s1 in segs:
                        nc.tensor.matmul(
                            pss[:, s0:s1],
                            mm(ktile[:, i * P:(i + 1) * P]),
                            mm(qtile[:, j * QCH + s0:j * QCH + s1]),
                            start=True,
                            stop=True,
                        )
                    nc.scalar.activation(
                        pt[:, vs:QCH],
                        pss[:, vs:QCH],
                        mybir.ActivationFunctionType.Exp,
                        scale=0.125,
                    )
                    if m >= 0:
                        nc.vector.tensor_mul(pt[:, vs:vs + P], pt[:, vs:vs + P], tri[:])
                    for half, s0, s1 in segs:
                        nc.tensor.matmul(
                            pso[:, s0:s1],
                            mm(v4_tiles[i][:, h * HDP:(h + 1) * HDP]),
                            mm(pt[:, s0:s1]),
                            start=(i == halves[half][0]),
                            stop=(i == halves[half][-1]),
                        )
                osb = ot_pool.tile([HD + 1, QCH], fp32, name="osb", tag="osb")
                nc.vector.tensor_copy(osb[:], pso[0:HD + 1, :])
                nc.sync.dma_start(
                    outT_d[h * (HD + 1):(h + 1) * (HD + 1), j * QCH:(j + 1) * QCH],
                    osb[:],
                )

            # dovetail attention (ACT-heavy) with remaining projection
            # (PE-only): every attn(h,0) needs only v4[0:8] + q_tiles[h//2]
            # + kp[h], so the rest of the projection is spread between the
            # j=0 chunks; attn(h,1) additionally needs v4[8:16].
            proj_qk(0)
            proj_qk(2)
            proj_v(range(0, 8))
            attn(0, 0)
            proj_qk(1)
            attn(1, 0)
            proj_qk(3)
            attn(2, 0)
            proj_v(range(8, KT))
            attn(3, 0)
            attn(0, 1)
            attn(1, 1)
            attn(2, 1)
            attn(3, 1)

    nc.compile()
    return nc


def _get_program():
    if "nc" not in _CACHE:
        _CACHE["nc"] = _build_program()
    return _CACHE["nc"]


def _in_maps(x, W_qkv, b_qkv):
    cdt_np = np.float32 if CMODE != "bf16" else None
    if cdt_np is None:
        import ml_dtypes

        cdt_np = ml_dtypes.bfloat16
    tri = np.triu(np.ones((P, P), np.float32)).astype(cdt_np)
    maps = []
    for c in range(NCORES):
        b, hg = divmod(c, GROUPS)
        r0 = hg * NV
        wq = W_qkv[r0:r0 + NV]
        wk = W_qkv[D + r0:D + r0 + NV]
        wvs = W_qkv[2 * D + r0:2 * D + r0 + NV]
        maps.append({
            "xT": np.ascontiguousarray(x[b].T).astype(cdt_np),
            "wqk": np.ascontiguousarray(np.concatenate([wq, wk], 0).T).astype(cdt_np),
            "wv": np.ascontiguousarray(wvs.T).astype(cdt_np),
            "bqk": np.ascontiguousarray(
                np.concatenate([b_qkv[r0:r0 + NV], b_qkv[D + r0:D + r0 + NV]])[:, None]
            ).astype(np.float32),
            "bv": np.broadcast_to(
                b_qkv[2 * D + r0:2 * D + r0 + NV], (P, NV)
            ).astype(np.float32),
            "tri": tri,
        })
    return maps


def kernel(x, W_qkv, b_qkv, _trace=False):
    x = np.asarray(x, np.float32)
    W_qkv = np.asarray(W_qkv, np.float32)
    b_qkv = np.asarray(b_qkv, np.float32)

    from concourse.bass_utils import run_bass_kernel_spmd

    nc = _get_program()
    maps = _in_maps(x, W_qkv, b_qkv)
    kw = {}
    if _trace:
        import tempfile

        kw["tmpdir"] = tempfile.mkdtemp(prefix="trn_trace_")
        _CACHE["trace_dir"] = kw["tmpdir"]
    res = run_bass_kernel_spmd(nc, maps, list(range(NCORES)), trace=_trace, **kw)
    _CACHE["last_result"] = res

    out = np.empty((BS, SEQ, D), np.float32)
    K = np.empty((BS, H, SEQ, HD), np.float32)
    V = np.empty((BS, H, SEQ, HD), np.float32)
    for c in range(NCORES):
        b, hg = divmod(c, GROUPS)
        r = res.results[c]
        kTo = np.asarray(r["kT"], np.float32)
        vo = np.asarray(r["v"], np.float32)
        oT = np.asarray(r["outT"], np.float32).reshape(HPG, HD + 1, SEQ)
        for hh in range(HPG):
            h = hg * HPG + hh
            K[b, h] = kTo[hh * HD:(hh + 1) * HD, :].T
            V[b, h] = vo[:, hh * HD:(hh + 1) * HD]
            out[b][:, h * HD:(h + 1) * HD] = (oT[hh, :HD, :] / oT[hh, HD, :]).T
    return (out, (K, V))


# revision 42
# speedup vs baseline: 1.0151x; 1.0151x over previous
# Causal multi-head self-attention (fused QKV) on 8 Trainium2 NeuronCores.
#
# Problem shapes (hardcoded): x [2, 2048, 1024], W_qkv [3072, 1024],
# b_qkv [3072]; H=16 heads, head_dim 64.
#
# Sharding: core c handles batch b = c//4 and head group hg = c%4 (4 heads).
# Each core:
#   - projects its 512 Q/K rows of W_qkv transposed   -> qkT [512, 2048]
#     (rows = (head, head_dim), cols = seq; i.e. Q^T / K^T per head)
#   - projects its 256 V rows non-transposed          -> v   [2048, 256]
#   - attention per head: S^T = K^T.T @ Q^T tiles (contraction over head
#     dim on partitions), exp on ACT with causal block skipping, then
#     out' = [V | 1]^T-style matmul so the softmax denominator falls out
#     of the same accumulation as the numerator (extra stationary column
#     of ones).  out'[65, seq] per head is DMA'd out unnormalized; the
#     host divides by row 64 and transposes during the gather.
#
# Matmuls run as float32r (full PE rate, fp32 operands) by default;
# set CMSA_CMODE=bf16 or f32 to switch.

import os

import numpy as np

BS, SEQ, D, H, HD = 2, 2048, 1024, 16, 64
NCORES = 8
GROUPS = NCORES // BS          # 4 head-groups per batch
HPG = H // GROUPS              # 4 heads per group/core
NQK = 2 * HPG * HD             # 512 rows of Q+K per core
NV = HPG * HD                  # 256 rows of V per core
P = 128
KT = SEQ // P                  # 16 k-tiles per head
QCH = 1024                     # q-chunk width (2 psum banks)
NJ = SEQ // QCH                # 2 q-chunks
DT_TILES = D // P              # 8 contraction tiles for the projection
HDP = HD + 2                   # per-head stride in v4: V | ones | zero-pad
                               # (even stationary width for the fp32r ISA rules)

CMODE = os.environ.get("CMSA_CMODE", "f32r")

_CACHE = {}


def _build_program():
    import concourse.mybir as mybir
    import concourse.tile as tile
    from concourse import bacc

    fp32 = mybir.dt.float32
    if CMODE == "bf16":
        cdt = mybir.dt.bfloat16
    elif CMODE == "f32":
        cdt = mybir.dt.float32
    else:
        # float32r: fp32-width operands at full PE rate; every producer of a
        # matmul operand must itself emit float32r, so the whole tile chain
        # (and the DRAM inputs they're DMA'd from) is typed float32r.
        cdt = mybir.dt.float32r

    def mm(ap):
        return ap

    nc = bacc.Bacc(
        "TRN2",
        target_bir_lowering=False,
        debug=False,
        enable_asserts=False,
        num_devices=NCORES,
    )

    # ---- DRAM I/O ----
    xT_d = nc.dram_tensor("xT", [D, SEQ], cdt, kind="ExternalInput").ap()
    wqk_d = nc.dram_tensor("wqk", [D, NQK], cdt, kind="ExternalInput").ap()
    wv_d = nc.dram_tensor("wv", [D, NV], cdt, kind="ExternalInput").ap()
    bqk_d = nc.dram_tensor("bqk", [NQK, 1], fp32, kind="ExternalInput").ap()
    bv_d = nc.dram_tensor("bv", [P, NV], fp32, kind="ExternalInput").ap()
    tri_d = nc.dram_tensor("tri", [P, P], cdt, kind="ExternalInput").ap()

    kT_d = nc.dram_tensor("kT", [NQK // 2, SEQ], cdt, kind="ExternalOutput").ap()
    v_d = nc.dram_tensor("v", [SEQ, NV], cdt, kind="ExternalOutput").ap()
    outT_d = nc.dram_tensor("outT", [HPG * (HD + 1), SEQ], fp32, kind="ExternalOutput").ap()

    with tile.TileContext(nc) as tc:
        from contextlib import ExitStack

        with ExitStack() as ctx:
            consts = ctx.enter_context(tc.tile_pool(name="consts", bufs=1))
            xt_pool = ctx.enter_context(tc.tile_pool(name="xt", bufs=1))
            w_pool = ctx.enter_context(tc.tile_pool(name="w", bufs=1))
            qk_pool = ctx.enter_context(tc.tile_pool(name="qk", bufs=1))
            v4_pool = ctx.enter_context(tc.tile_pool(name="v4", bufs=1))
            pt_pool = ctx.enter_context(tc.tile_pool(name="pt", bufs=4))
            ot_pool = ctx.enter_context(tc.tile_pool(name="ot", bufs=2))
            # PSUM budget (8 banks): scores 2x[128,1024] (4) + proj
            # 2x[128,512] (2) + PV accumulator 1x[66,1024] (2)
            psA = ctx.enter_context(tc.tile_pool(name="psA", bufs=2, space="PSUM"))
            psP = ctx.enter_context(tc.tile_pool(name="psP", bufs=2, space="PSUM"))
            psB = ctx.enter_context(tc.tile_pool(name="psB", bufs=1, space="PSUM"))

            # ---- load inputs ----
            # (wqk_d, xt_d) pairs first so the projection's d-accumulation can
            # start as soon as the first pair lands rather than after the full
            # 11MB input load.
            tri = consts.tile([P, P], cdt, name="tri", tag="tri")
            nc.sync.dma_start(tri[:], tri_d[:, :])
            bqk = consts.tile([P, NQK // P], fp32, name="bqk", tag="bqk")
            for nt in range(NQK // P):
                nc.sync.dma_start(bqk[:, nt:nt + 1], bqk_d[nt * P:(nt + 1) * P, :])
            bv = consts.tile([P, NV], fp32, name="bv", tag="bv")
            nc.sync.dma_start(bv[:], bv_d[:, :])
            xt = [xt_pool.tile([P, SEQ], cdt, name=f"xt{d}", tag=f"xt{d}") for d in range(DT_TILES)]
            wqk = [w_pool.tile([P, NQK], cdt, name=f"wqk{d}", tag=f"wqk{d}") for d in range(DT_TILES)]
            wv = [w_pool.tile([P, NV], cdt, name=f"wv{d}", tag=f"wv{d}") for d in range(DT_TILES)]
            # wqk tiles whole (contiguous rows, only 2MB total), xt in
            # column quarters delivered in consumption order so the first
            # projection chunk starts after ~4MB instead of the full 10MB.
            for d in range(DT_TILES):
                nc.sync.dma_start(wqk[d][:], wqk_d[d * P:(d + 1) * P, :])
            for blk in range(4):
                for d in range(DT_TILES):
                    nc.sync.dma_start(
                        xt[d][:, blk * 512:(blk + 1) * 512],
                        xT_d[d * P:(d + 1) * P, blk * 512:(blk + 1) * 512],
                    )
            for d in range(DT_TILES):
                nc.sync.dma_start(wv[d][:], wv_d[d * P:(d + 1) * P, :])
            # fp32 constant tiles (gpsimd memset can't write float32r; DVE
            # copies from these do the fp32 -> fp32r rounding instead)
            oz = consts.tile([P, 2 * HPG], fp32, name="oz", tag="oz")
            ozs = oz[:].rearrange("p (h e) -> p h e", h=HPG)
            nc.gpsimd.memset(ozs[:, :, 0:1], 1.0)
            nc.gpsimd.memset(ozs[:, :, 1:2], 0.0)
            zer = consts.tile([P, SEQ], fp32, name="zer", tag="zer")
            nc.gpsimd.memset(zer[:], 0.0)

            # ---- projection: qkT[n, s] = sum_d wqk[d, n] * xT[d, s] + bqk[n]
            # Q^T stays packed two-heads-per-tile (it is the 128-row moving
            # operand of the scores matmul).  K^T goes into per-head tiles
            # zero-padded to 128 partitions, so the scores stationary is a
            # full (128,128) tile — same PE config as every other matmul
            # (the pad rows hit the odd head's Q rows and contribute 0).
            # K^T of head h sits at rows 64*(h%2)..+64 (aligned with that
            # head's Q^T rows in the packed q tile); the other 64 rows are 0.
            q_tiles = [qk_pool.tile([P, SEQ], cdt, name=f"q{nt}", tag=f"q{nt}") for nt in range(2)]
            kp_tiles = [qk_pool.tile([P, SEQ], cdt, name=f"kp{h}", tag=f"kp{h}") for h in range(HPG)]
            for h in range(HPG):
                zrow = 64 * (1 - h % 2)
                nc.vector.tensor_copy(kp_tiles[h][zrow:zrow + 64, :], zer[0:64, :])

            def proj_qk(nt):
                for sc in range(4):
                    o = sc * 512
                    ps = psP.tile([P, 512], fp32, name="psP", tag="psP")
                    for d in range(DT_TILES):
                        nc.tensor.matmul(
                            ps[:],
                            mm(wqk[d][:, nt * P:(nt + 1) * P]),
                            mm(xt[d][:, o:o + 512]),
                            start=(d == 0),
                            stop=(d == DT_TILES - 1),
                        )
                    if nt < 2:
                        nc.vector.tensor_scalar_add(
                            q_tiles[nt][:, o:o + 512], ps[:], bqk[:, nt:nt + 1]
                        )
                    else:
                        for hh in range(2):
                            h = (nt - 2) * 2 + hh
                            r = hh * 64
                            nc.vector.tensor_scalar_add(
                                kp_tiles[h][r:r + 64, o:o + 512],
                                ps[r:r + 64, :],
                                bqk[r:r + 64, nt:nt + 1],
                            )
                            nc.sync.dma_start(
                                kT_d[h * 64:(h + 1) * 64, o:o + 512],
                                kp_tiles[h][r:r + 64, o:o + 512],
                            )

            # ---- projection: v[s, n] = sum_d xT[d, s] * wv[d, n] + bv[n]
            # v4 tiles hold [V | 1] interleaved per head: cols h*65..h*65+63 = V,
            # col h*65+64 = ones (PV stationary with free denominator row).
            v4_tiles = []

            def proj_v(sts):
                for st in sts:
                    ps = psP.tile([P, 512], fp32, name="psP", tag="psP")
                    for d in range(DT_TILES):
                        nc.tensor.matmul(
                            ps[:, 0:NV],
                            mm(xt[d][:, st * P:(st + 1) * P]),
                            mm(wv[d][:]),
                            start=(d == 0),
                            stop=(d == DT_TILES - 1),
                        )
                    v4 = v4_pool.tile([P, HPG * HDP], cdt, name=f"v4_{st}", tag=f"v4_{st}")
                    v4r = v4[:].rearrange("p (h e) -> p h e", h=HPG)
                    v4s = v4r[:, :, 0:HD]
                    nc.vector.tensor_copy(v4r[:, :, HD:HDP], ozs)
                    nc.vector.tensor_add(
                        v4s,
                        ps[:, 0:NV].rearrange("p (h e) -> p h e", h=HPG),
                        bv[:].rearrange("p (h e) -> p h e", h=HPG),
                    )
                    nc.sync.dma_start(
                        v_d[st * P:(st + 1) * P, :].rearrange("p (h e) -> p h e", h=HPG),
                        v4s,
                    )
                    v4_tiles.append(v4)

            # ---- attention for one head, one q-chunk ----
            def attn(h, j):
                ktile = kp_tiles[h]
                qtile = q_tiles[h // 2]
                n_k = 8 * (j + 1)
                # diagonal k-tiles first: their serial exp chains overlap the
                # full-tile matmuls instead of sitting on the kernel tail
                order = list(range(8 * j, n_k)) + list(range(0, 8 * j))
                halves = {0: [i for i in order if i - 8 * j < 4], 1: order}
                pso = psB.tile([HD + 2, QCH], fp32, name="pso", tag="pso")
                for i in order:
                    m = i - 8 * j  # >= 0 on the diagonal band
                    vs = max(P * m, 0)  # first causally-valid q col in chunk
                    pss = psA.tile([P, QCH], fp32, name="psA", tag="psA")
                    pt = pt_pool.tile([P, QCH], cdt, name="pt", tag="pt")
                    # per 512-half, only the causally valid segment [s0:s1):
                    # scores/PV stream just that range, so the masked-out
                    # left part of pt is never written NOR read.
                    segs = []
                    for half in range(2):
                        s0, s1 = max(half * 512, vs), (half + 1) * 512
                        if s0 < s1:
                            segs.append((half, s0, s1))
                    for half, s0, s1 in segs:
                        nc.tensor.matmul(
                            pss[:, s0:s1],
                            mm(ktile[:, i * P:(i + 1) * P]),
                            mm(qtile[:, j * QCH + s0:j * QCH + s1]),
                            start=True,
                            stop=True,
                        )
                    nc.scalar.activation(
                        pt[:, vs:QCH],
                        pss[:, vs:QCH],
                        mybir.ActivationFunctionType.Exp,
                        scale=0.125,
                    )
                    if m >= 0:
                        nc.vector.tensor_mul(pt[:, vs:vs + P], pt[:, vs:vs + P], tri[:])
                    for half, s0, s1 in segs:
                        nc.tensor.matmul(
                            pso[:, s0:s1],
                            mm(v4_tiles[i][:, h * HDP:(h + 1) * HDP]),
                            mm(pt[:, s0:s1]),
                            start=(i == halves[half][0]),
                            stop=(i == halves[half][-1]),
                        )
                osb = ot_pool.tile([HD + 1, QCH], fp32, name="osb", tag="osb")
                nc.vector.tensor_copy(osb[:], pso[0:HD + 1, :])
                nc.sync.dma_start(
                    outT_d[h * (HD + 1):(h + 1) * (HD + 1), j * QCH:(j + 1) * QCH],
                    osb[:],
                )

            # dovetail attention (ACT-heavy) with remaining projection
            # (PE-only): every attn(h,0) needs only v4[0:8] + q_tiles[h//2]
            # + kp[h], so the rest of the projection is spread between the
            # j=0 chunks; attn(h,1) additionally needs v4[8:16].
            proj_qk(0)
            proj_qk(2)
            proj_v(range(0, 8))
            attn(0, 0)
            proj_qk(1)
            attn(1, 0)
            proj_qk(3)
            attn(2, 0)
            proj_v(range(8, KT))
            attn(3, 0)
            attn(0, 1)
            attn(1, 1)
            attn(2, 1)
            attn(3, 1)

    nc.compile()
    return nc


def _get_program():
    if "nc" not in _CACHE:
        _CACHE["nc"] = _build_program()
    return _CACHE["nc"]


def _in_maps(x, W_qkv, b_qkv):
    cdt_np = np.float32 if CMODE != "bf16" else None
    if cdt_np is None:
        import ml_dtypes

        cdt_np = ml_dtypes.bfloat16
    tri = np.triu(np.ones((P, P), np.float32)).astype(cdt_np)
    maps = []
    for c in range(NCORES):
        b, hg = divmod(c, GROUPS)
        r0 = hg * NV
        wq = W_qkv[r0:r0 + NV]
        wk = W_qkv[D + r0:D + r0 + NV]
        wvs = W_qkv[2 * D + r0:2 * D + r0 + NV]
        maps.append({
            "xT": np.ascontiguousarray(x[b].T).astype(cdt_np),
            "wqk": np.ascontiguousarray(np.concatenate([wq, wk], 0).T).astype(cdt_np),
            "wv": np.ascontiguousarray(wvs.T).astype(cdt_np),
            "bqk": np.ascontiguousarray(
                np.concatenate([b_qkv[r0:r0 + NV], b_qkv[D + r0:D + r0 + NV]])[:, None]
            ).astype(np.float32),
            "bv": np.broadcast_to(
                b_qkv[2 * D + r0:2 * D + r0 + NV], (P, NV)
            ).astype(np.float32),
            "tri": tri,
        })
    return maps


def kernel(x, W_qkv, b_qkv, _trace=False):
    x = np.asarray(x, np.float32)
    W_qkv = np.asarray(W_qkv, np.float32)
    b_qkv = np.asarray(b_qkv, np.float32)

    from concourse.bass_utils import run_bass_kernel_spmd

    nc = _get_program()
    maps = _in_maps(x, W_qkv, b_qkv)
    kw = {}
    if _trace:
        import tempfile

        kw["tmpdir"] = tempfile.mkdtemp(prefix="trn_trace_")
        _CACHE["trace_dir"] = kw["tmpdir"]
    res = run_bass_kernel_spmd(nc, maps, list(range(NCORES)), trace=_trace, **kw)
    _CACHE["last_result"] = res

    out = np.empty((BS, SEQ, D), np.float32)
    K = np.empty((BS, H, SEQ, HD), np.float32)
    V = np.empty((BS, H, SEQ, HD), np.float32)
    for c in range(NCORES):
        b, hg = divmod(c, GROUPS)
        r = res.results[c]
        kTo = np.asarray(r["kT"], np.float32)
        vo = np.asarray(r["v"], np.float32)
        oT = np.asarray(r["outT"], np.float32).reshape(HPG, HD + 1, SEQ)
        for hh in range(HPG):
            h = hg * HPG + hh
            K[b, h] = kTo[hh * HD:(hh + 1) * HD, :].T
            V[b, h] = vo[:, hh * HD:(hh + 1) * HD]
            out[b][:, h * HD:(h + 1) * HD] = (oT[hh, :HD, :] / oT[hh, HD, :]).T
    return (out, (K, V))


# revision 45
# speedup vs baseline: 1.0445x; 1.0289x over previous
# Causal multi-head self-attention (fused QKV) on 8 Trainium2 NeuronCores.
#
# Problem shapes (hardcoded): x [2, 2048, 1024], W_qkv [3072, 1024],
# b_qkv [3072]; H=16 heads, head_dim 64.
#
# Sharding: core c handles batch b = c//4 and head group hg = c%4 (4 heads).
# Each core:
#   - projects its 512 Q/K rows of W_qkv transposed   -> qkT [512, 2048]
#     (rows = (head, head_dim), cols = seq; i.e. Q^T / K^T per head)
#   - projects its 256 V rows non-transposed          -> v   [2048, 256]
#   - attention per head: S^T = K^T.T @ Q^T tiles (contraction over head
#     dim on partitions), exp on ACT with causal block skipping, then
#     out' = [V | 1]^T-style matmul so the softmax denominator falls out
#     of the same accumulation as the numerator (extra stationary column
#     of ones).  out'[65, seq] per head is DMA'd out unnormalized; the
#     host divides by row 64 and transposes during the gather.
#
# Matmuls run as float32r (full PE rate, fp32 operands) by default;
# set CMSA_CMODE=bf16 or f32 to switch.

import os

import numpy as np

BS, SEQ, D, H, HD = 2, 2048, 1024, 16, 64
NCORES = 8
GROUPS = NCORES // BS          # 4 head-groups per batch
HPG = H // GROUPS              # 4 heads per group/core
NQK = 2 * HPG * HD             # 512 rows of Q+K per core
NV = HPG * HD                  # 256 rows of V per core
P = 128
KT = SEQ // P                  # 16 k-tiles per head
QCH = 1024                     # q-chunk width (2 psum banks)
NJ = SEQ // QCH                # 2 q-chunks
DT_TILES = D // P              # 8 contraction tiles for the projection
HDP = HD + 2                   # per-head stride in v4: V | ones | zero-pad
                               # (even stationary width for the fp32r ISA rules)

CMODE = os.environ.get("CMSA_CMODE", "f32r")

_CACHE = {}


def _build_program():
    import concourse.mybir as mybir
    import concourse.tile as tile
    from concourse import bacc

    fp32 = mybir.dt.float32
    if CMODE == "bf16":
        cdt = mybir.dt.bfloat16
    elif CMODE == "f32":
        cdt = mybir.dt.float32
    else:
        # float32r: fp32-width operands at full PE rate; every producer of a
        # matmul operand must itself emit float32r, so the whole tile chain
        # (and the DRAM inputs they're DMA'd from) is typed float32r.
        cdt = mybir.dt.float32r

    def mm(ap):
        return ap

    nc = bacc.Bacc(
        "TRN2",
        target_bir_lowering=False,
        debug=False,
        enable_asserts=False,
        num_devices=NCORES,
    )

    # ---- DRAM I/O ----
    xT_d = nc.dram_tensor("xT", [D, SEQ], cdt, kind="ExternalInput").ap()
    wqk_d = nc.dram_tensor("wqk", [D, NQK], cdt, kind="ExternalInput").ap()
    wv_d = nc.dram_tensor("wv", [D, NV], cdt, kind="ExternalInput").ap()
    bqk_d = nc.dram_tensor("bqk", [NQK, 1], fp32, kind="ExternalInput").ap()
    bv_d = nc.dram_tensor("bv", [P, NV], fp32, kind="ExternalInput").ap()
    tri_d = nc.dram_tensor("tri", [P, P], cdt, kind="ExternalInput").ap()

    kT_d = nc.dram_tensor("kT", [NQK // 2, SEQ], cdt, kind="ExternalOutput").ap()
    v_d = nc.dram_tensor("v", [SEQ, NV], cdt, kind="ExternalOutput").ap()
    outT_d = nc.dram_tensor("outT", [HPG * (HD + 1), SEQ], fp32, kind="ExternalOutput").ap()

    with tile.TileContext(nc) as tc:
        from contextlib import ExitStack

        with ExitStack() as ctx:
            consts = ctx.enter_context(tc.tile_pool(name="consts", bufs=1))
            xt_pool = ctx.enter_context(tc.tile_pool(name="xt", bufs=1))
            w_pool = ctx.enter_context(tc.tile_pool(name="w", bufs=1))
            qk_pool = ctx.enter_context(tc.tile_pool(name="qk", bufs=1))
            v4_pool = ctx.enter_context(tc.tile_pool(name="v4", bufs=1))
            pt_pool = ctx.enter_context(tc.tile_pool(name="pt", bufs=5))
            ot_pool = ctx.enter_context(tc.tile_pool(name="ot", bufs=2))
            # PSUM budget (8 banks): scores 2x[128,1024] (4) + proj
            # 2x[128,512] (2) + PV accumulator 1x[66,1024] (2)
            psA = ctx.enter_context(tc.tile_pool(name="psA", bufs=2, space="PSUM"))
            psP = ctx.enter_context(tc.tile_pool(name="psP", bufs=2, space="PSUM"))
            psB = ctx.enter_context(tc.tile_pool(name="psB", bufs=1, space="PSUM"))

            # ---- load inputs ----
            # (wqk_d, xt_d) pairs first so the projection's d-accumulation can
            # start as soon as the first pair lands rather than after the full
            # 11MB input load.
            tri = consts.tile([P, P], cdt, name="tri", tag="tri")
            nc.sync.dma_start(tri[:], tri_d[:, :])
            bqk = consts.tile([P, NQK // P], fp32, name="bqk", tag="bqk")
            for nt in range(NQK // P):
                nc.sync.dma_start(bqk[:, nt:nt + 1], bqk_d[nt * P:(nt + 1) * P, :])
            bv = consts.tile([P, NV], fp32, name="bv", tag="bv")
            nc.sync.dma_start(bv[:], bv_d[:, :])
            xt = [xt_pool.tile([P, SEQ], cdt, name=f"xt{d}", tag=f"xt{d}") for d in range(DT_TILES)]
            wqk = [w_pool.tile([P, NQK], cdt, name=f"wqk{d}", tag=f"wqk{d}") for d in range(DT_TILES)]
            wv = [w_pool.tile([P, NV], cdt, name=f"wv{d}", tag=f"wv{d}") for d in range(DT_TILES)]
            # wqk tiles whole (contiguous rows, only 2MB total), xt in
            # column quarters delivered in consumption order so the first
            # projection chunk starts after ~4MB instead of the full 10MB.
            for d in range(DT_TILES):
                nc.sync.dma_start(wqk[d][:], wqk_d[d * P:(d + 1) * P, :])
            for blk in range(4):
                for d in range(DT_TILES):
                    nc.sync.dma_start(
                        xt[d][:, blk * 512:(blk + 1) * 512],
                        xT_d[d * P:(d + 1) * P, blk * 512:(blk + 1) * 512],
                    )
            for d in range(DT_TILES):
                nc.sync.dma_start(wv[d][:], wv_d[d * P:(d + 1) * P, :])
            # fp32 constant tiles (gpsimd memset can't write float32r; DVE
            # copies from these do the fp32 -> fp32r rounding instead)
            oz = consts.tile([P, 2 * HPG], fp32, name="oz", tag="oz")
            ozs = oz[:].rearrange("p (h e) -> p h e", h=HPG)
            nc.gpsimd.memset(ozs[:, :, 0:1], 1.0)
            nc.gpsimd.memset(ozs[:, :, 1:2], 0.0)
            zer = consts.tile([P, SEQ], fp32, name="zer", tag="zer")
            nc.gpsimd.memset(zer[:], 0.0)

            # ---- projection: qkT[n, s] = sum_d wqk[d, n] * xT[d, s] + bqk[n]
            # Q^T stays packed two-heads-per-tile (it is the 128-row moving
            # operand of the scores matmul).  K^T goes into per-head tiles
            # zero-padded to 128 partitions, so the scores stationary is a
            # full (128,128) tile — same PE config as every other matmul
            # (the pad rows hit the odd head's Q rows and contribute 0).
            # K^T of head h sits at rows 64*(h%2)..+64 (aligned with that
            # head's Q^T rows in the packed q tile); the other 64 rows are 0.
            q_tiles = [qk_pool.tile([P, SEQ], cdt, name=f"q{nt}", tag=f"q{nt}") for nt in range(2)]
            kp_tiles = [qk_pool.tile([P, SEQ], cdt, name=f"kp{h}", tag=f"kp{h}") for h in range(HPG)]
            for h in range(HPG):
                zrow = 64 * (1 - h % 2)
                nc.vector.tensor_copy(kp_tiles[h][zrow:zrow + 64, :], zer[0:64, :])

            def proj_qk(nt):
                for sc in range(4):
                    o = sc * 512
                    ps = psP.tile([P, 512], fp32, name="psP", tag="psP")
                    for d in range(DT_TILES):
                        nc.tensor.matmul(
                            ps[:],
                            mm(wqk[d][:, nt * P:(nt + 1) * P]),
                            mm(xt[d][:, o:o + 512]),
                            start=(d == 0),
                            stop=(d == DT_TILES - 1),
                        )
                    if nt < 2:
                        nc.vector.tensor_scalar_add(
                            q_tiles[nt][:, o:o + 512], ps[:], bqk[:, nt:nt + 1]
                        )
                    else:
                        for hh in range(2):
                            h = (nt - 2) * 2 + hh
                            r = hh * 64
                            nc.vector.tensor_scalar_add(
                                kp_tiles[h][r:r + 64, o:o + 512],
                                ps[r:r + 64, :],
                                bqk[r:r + 64, nt:nt + 1],
                            )
                            nc.sync.dma_start(
                                kT_d[h * 64:(h + 1) * 64, o:o + 512],
                                kp_tiles[h][r:r + 64, o:o + 512],
                            )

            # ---- projection: v[s, n] = sum_d xT[d, s] * wv[d, n] + bv[n]
            # v4 tiles hold [V | 1] interleaved per head: cols h*65..h*65+63 = V,
            # col h*65+64 = ones (PV stationary with free denominator row).
            v4_tiles = []

            def proj_v(sts):
                for st in sts:
                    ps = psP.tile([P, 512], fp32, name="psP", tag="psP")
                    for d in range(DT_TILES):
                        nc.tensor.matmul(
                            ps[:, 0:NV],
                            mm(xt[d][:, st * P:(st + 1) * P]),
                            mm(wv[d][:]),
                            start=(d == 0),
                            stop=(d == DT_TILES - 1),
                        )
                    v4 = v4_pool.tile([P, HPG * HDP], cdt, name=f"v4_{st}", tag=f"v4_{st}")
                    v4r = v4[:].rearrange("p (h e) -> p h e", h=HPG)
                    v4s = v4r[:, :, 0:HD]
                    nc.vector.tensor_copy(v4r[:, :, HD:HDP], ozs)
                    nc.vector.tensor_add(
                        v4s,
                        ps[:, 0:NV].rearrange("p (h e) -> p h e", h=HPG),
                        bv[:].rearrange("p (h e) -> p h e", h=HPG),
                    )
                    nc.sync.dma_start(
                        v_d[st * P:(st + 1) * P, :].rearrange("p (h e) -> p h e", h=HPG),
                        v4s,
                    )
                    v4_tiles.append(v4)

            # ---- attention for one head, one q-chunk ----
            def attn(h, j, last=False):
                ktile = kp_tiles[h]
                qtile = q_tiles[h // 2]
                n_k = 8 * (j + 1)
                # diagonal k-tiles first: their serial exp chains overlap the
                # full-tile matmuls instead of sitting on the kernel tail.
                # The very last chunk instead ends on the SMALLEST diagonal
                # exp (m=7, 128 cols) so the closing serial chain is short.
                if last:
                    order = list(range(0, 8 * j)) + list(range(8 * j, n_k))
                else:
                    order = list(range(8 * j, n_k)) + list(range(0, 8 * j))
                halves = {0: [i for i in order if i - 8 * j < 4], 1: order}
                pso = psB.tile([HD + 2, QCH], fp32, name="pso", tag="pso")
                for i in order:
                    m = i - 8 * j  # >= 0 on the diagonal band
                    vs = max(P * m, 0)  # first causally-valid q col in chunk
                    pss = psA.tile([P, QCH], fp32, name="psA", tag="psA")
                    pt = pt_pool.tile([P, QCH], cdt, name="pt", tag="pt")
                    # per 512-half, only the causally valid segment [s0:s1):
                    # scores/PV stream just that range, so the masked-out
                    # left part of pt is never written NOR read.
                    segs = []
                    for half in range(2):
                        s0, s1 = max(half * 512, vs), (half + 1) * 512
                        if s0 < s1:
                            segs.append((half, s0, s1))
                    for half, s0, s1 in segs:
                        nc.tensor.matmul(
                            pss[:, s0:s1],
                            mm(ktile[:, i * P:(i + 1) * P]),
                            mm(qtile[:, j * QCH + s0:j * QCH + s1]),
                            start=True,
                            stop=True,
                        )
                    nc.scalar.activation(
                        pt[:, vs:QCH],
                        pss[:, vs:QCH],
                        mybir.ActivationFunctionType.Exp,
                        scale=0.125,
                    )
                    if m >= 0:
                        nc.vector.tensor_mul(pt[:, vs:vs + P], pt[:, vs:vs + P], tri[:])
                    for half, s0, s1 in segs:
                        nc.tensor.matmul(
                            pso[:, s0:s1],
                            mm(v4_tiles[i][:, h * HDP:(h + 1) * HDP]),
                            mm(pt[:, s0:s1]),
                            start=(i == halves[half][0]),
                            stop=(i == halves[half][-1]),
                        )
                osb = ot_pool.tile([HD + 1, QCH], fp32, name="osb", tag="osb")
                nc.vector.tensor_copy(osb[:], pso[0:HD + 1, :])
                nc.sync.dma_start(
                    outT_d[h * (HD + 1):(h + 1) * (HD + 1), j * QCH:(j + 1) * QCH],
                    osb[:],
                )

            # dovetail attention (ACT-heavy) with remaining projection
            # (PE-only): every attn(h,0) needs only v4[0:8] + q_tiles[h//2]
            # + kp[h], so the rest of the projection is spread between the
            # j=0 chunks; attn(h,1) additionally needs v4[8:16].
            proj_qk(0)
            proj_qk(2)
            proj_v(range(0, 8))
            attn(0, 0)
            proj_qk(1)
            attn(1, 0)
            proj_qk(3)
            attn(2, 0)
            proj_v(range(8, KT))
            attn(3, 0)
            attn(0, 1)
            attn(1, 1)
            attn(2, 1)
            attn(3, 1, last=True)

    nc.compile()
    return nc


def _get_program():
    if "nc" not in _CACHE:
        _CACHE["nc"] = _build_program()
    return _CACHE["nc"]


def _in_maps(x, W_qkv, b_qkv):
    cdt_np = np.float32 if CMODE != "bf16" else None
    if cdt_np is None:
        import ml_dtypes

        cdt_np = ml_dtypes.bfloat16
    tri = np.triu(np.ones((P, P), np.float32)).astype(cdt_np)
    maps = []
    for c in range(NCORES):
        b, hg = divmod(c, GROUPS)
        r0 = hg * NV
        wq = W_qkv[r0:r0 + NV]
        wk = W_qkv[D + r0:D + r0 + NV]
        wvs = W_qkv[2 * D + r0:2 * D + r0 + NV]
        maps.append({
            "xT": np.ascontiguousarray(x[b].T).astype(cdt_np),
            "wqk": np.ascontiguousarray(np.concatenate([wq, wk], 0).T).astype(cdt_np),
            "wv": np.ascontiguousarray(wvs.T).astype(cdt_np),
            "bqk": np.ascontiguousarray(
                np.concatenate([b_qkv[r0:r0 + NV], b_qkv[D + r0:D + r0 + NV]])[:, None]
            ).astype(np.float32),
            "bv": np.broadcast_to(
                b_qkv[2 * D + r0:2 * D + r0 + NV], (P, NV)
            ).astype(np.float32),
            "tri": tri,
        })
    return maps


def kernel(x, W_qkv, b_qkv, _trace=False):
    x = np.asarray(x, np.float32)
    W_qkv = np.asarray(W_qkv, np.float32)
    b_qkv = np.asarray(b_qkv, np.float32)

    from concourse.bass_utils import run_bass_kernel_spmd

    nc = _get_program()
    maps = _in_maps(x, W_qkv, b_qkv)
    kw = {}
    if _trace:
        import tempfile

        kw["tmpdir"] = tempfile.mkdtemp(prefix="trn_trace_")
        _CACHE["trace_dir"] = kw["tmpdir"]
    res = run_bass_kernel_spmd(nc, maps, list(range(NCORES)), trace=_trace, **kw)
    _CACHE["last_result"] = res

    out = np.empty((BS, SEQ, D), np.float32)
    K = np.empty((BS, H, SEQ, HD), np.float32)
    V = np.empty((BS, H, SEQ, HD), np.float32)
    for c in range(NCORES):
        b, hg = divmod(c, GROUPS)
        r = res.results[c]
        kTo = np.asarray(r["kT"], np.float32)
        vo = np.asarray(r["v"], np.float32)
        oT = np.asarray(r["outT"], np.float32).reshape(HPG, HD + 1, SEQ)
        for hh in range(HPG):
            h = hg * HPG + hh
            K[b, h] = kTo[hh * HD:(hh + 1) * HD, :].T
            V[b, h] = vo[:, hh * HD:(hh + 1) * HD]
            out[b][:, h * HD:(h + 1) * HD] = (oT[hh, :HD, :] / oT[hh, HD, :]).T
    return (out, (K, V))


# revision 49
# speedup vs baseline: 1.0527x; 1.0079x over previous
# Causal multi-head self-attention (fused QKV) on 8 Trainium2 NeuronCores.
#
# Problem shapes (hardcoded): x [2, 2048, 1024], W_qkv [3072, 1024],
# b_qkv [3072]; H=16 heads, head_dim 64.
#
# Sharding: core c handles batch b = c//4 and head group hg = c%4 (4 heads).
# Each core:
#   - projects its 512 Q/K rows of W_qkv transposed   -> qkT [512, 2048]
#     (rows = (head, head_dim), cols = seq; i.e. Q^T / K^T per head)
#   - projects its 256 V rows non-transposed          -> v   [2048, 256]
#   - attention per head: S^T = K^T.T @ Q^T tiles (contraction over head
#     dim on partitions), exp on ACT with causal block skipping, then
#     out' = [V | 1]^T-style matmul so the softmax denominator falls out
#     of the same accumulation as the numerator (extra stationary column
#     of ones).  out'[65, seq] per head is DMA'd out unnormalized; the
#     host divides by row 64 and transposes during the gather.
#
# Matmuls run as float32r (full PE rate, fp32 operands) by default;
# set CMSA_CMODE=bf16 or f32 to switch.

import os

import numpy as np

BS, SEQ, D, H, HD = 2, 2048, 1024, 16, 64
NCORES = 8
GROUPS = NCORES // BS          # 4 head-groups per batch
HPG = H // GROUPS              # 4 heads per group/core
NQK = 2 * HPG * HD             # 512 rows of Q+K per core
NV = HPG * HD                  # 256 rows of V per core
P = 128
KT = SEQ // P                  # 16 k-tiles per head
QCH = 1024                     # q-chunk width (2 psum banks)
NJ = SEQ // QCH                # 2 q-chunks
DT_TILES = D // P              # 8 contraction tiles for the projection
HDP = HD + 2                   # per-head stride in v4: V | ones | zero-pad
                               # (even stationary width for the fp32r ISA rules)

CMODE = os.environ.get("CMSA_CMODE", "f32r")

_CACHE = {}


def _build_program():
    import concourse.mybir as mybir
    import concourse.tile as tile
    from concourse import bacc

    fp32 = mybir.dt.float32
    if CMODE == "bf16":
        cdt = mybir.dt.bfloat16
    elif CMODE == "f32":
        cdt = mybir.dt.float32
    else:
        # float32r: fp32-width operands at full PE rate; every producer of a
        # matmul operand must itself emit float32r, so the whole tile chain
        # (and the DRAM inputs they're DMA'd from) is typed float32r.
        cdt = mybir.dt.float32r

    def mm(ap):
        return ap

    nc = bacc.Bacc(
        "TRN2",
        target_bir_lowering=False,
        debug=False,
        enable_asserts=False,
        num_devices=NCORES,
    )

    # ---- DRAM I/O ----
    xT_d = nc.dram_tensor("xT", [D, SEQ], cdt, kind="ExternalInput").ap()
    wqk_d = nc.dram_tensor("wqk", [D, NQK], cdt, kind="ExternalInput").ap()
    wv_d = nc.dram_tensor("wv", [D, NV], cdt, kind="ExternalInput").ap()
    bqk_d = nc.dram_tensor("bqk", [NQK, 1], fp32, kind="ExternalInput").ap()
    bv_d = nc.dram_tensor("bv", [P, NV], fp32, kind="ExternalInput").ap()
    tri_d = nc.dram_tensor("tri", [P, P], cdt, kind="ExternalInput").ap()

    kT_d = nc.dram_tensor("kT", [NQK // 2, SEQ], cdt, kind="ExternalOutput").ap()
    v_d = nc.dram_tensor("v", [SEQ, NV], cdt, kind="ExternalOutput").ap()
    outT_d = nc.dram_tensor("outT", [HPG * (HD + 1), SEQ], fp32, kind="ExternalOutput").ap()

    with tile.TileContext(nc) as tc:
        from contextlib import ExitStack

        with ExitStack() as ctx:
            consts = ctx.enter_context(tc.tile_pool(name="consts", bufs=1))
            xt_pool = ctx.enter_context(tc.tile_pool(name="xt", bufs=1))
            w_pool = ctx.enter_context(tc.tile_pool(name="w", bufs=1))
            qk_pool = ctx.enter_context(tc.tile_pool(name="qk", bufs=1))
            v4_pool = ctx.enter_context(tc.tile_pool(name="v4", bufs=1))
            pt_pool = ctx.enter_context(tc.tile_pool(name="pt", bufs=5))
            ot_pool = ctx.enter_context(tc.tile_pool(name="ot", bufs=2))
            # PSUM budget (8 banks): scores 2x[128,1024] (4) + proj
            # 2x[128,512] (2) + PV accumulator 1x[66,1024] (2)
            psA = ctx.enter_context(tc.tile_pool(name="psA", bufs=2, space="PSUM"))
            psP = ctx.enter_context(tc.tile_pool(name="psP", bufs=2, space="PSUM"))
            psB = ctx.enter_context(tc.tile_pool(name="psB", bufs=1, space="PSUM"))

            # ---- load inputs ----
            # (wqk_d, xt_d) pairs first so the projection's d-accumulation can
            # start as soon as the first pair lands rather than after the full
            # 11MB input load.
            tri = consts.tile([P, P], cdt, name="tri", tag="tri")
            nc.sync.dma_start(tri[:], tri_d[:, :])
            bqk = consts.tile([P, NQK // P], fp32, name="bqk", tag="bqk")
            for nt in range(NQK // P):
                nc.sync.dma_start(bqk[:, nt:nt + 1], bqk_d[nt * P:(nt + 1) * P, :])
            bv = consts.tile([P, NV], fp32, name="bv", tag="bv")
            nc.sync.dma_start(bv[:], bv_d[:, :])
            xt = [xt_pool.tile([P, SEQ], cdt, name=f"xt{d}", tag=f"xt{d}") for d in range(DT_TILES)]
            wqk = [w_pool.tile([P, NQK], cdt, name=f"wqk{d}", tag=f"wqk{d}") for d in range(DT_TILES)]
            wv = [w_pool.tile([P, NV], cdt, name=f"wv{d}", tag=f"wv{d}") for d in range(DT_TILES)]
            # wqk tiles whole (contiguous rows, only 2MB total), xt in
            # column quarters delivered in consumption order so the first
            # projection chunk starts after ~4MB instead of the full 10MB.
            for d in range(DT_TILES):
                nc.sync.dma_start(wqk[d][:], wqk_d[d * P:(d + 1) * P, :])
            for blk in range(4):
                for d in range(DT_TILES):
                    nc.sync.dma_start(
                        xt[d][:, blk * 512:(blk + 1) * 512],
                        xT_d[d * P:(d + 1) * P, blk * 512:(blk + 1) * 512],
                    )
            for d in range(DT_TILES):
                nc.sync.dma_start(wv[d][:], wv_d[d * P:(d + 1) * P, :])
            # fp32 constant tiles (gpsimd memset can't write float32r; DVE
            # copies from these do the fp32 -> fp32r rounding instead)
            oz = consts.tile([P, 2 * HPG], fp32, name="oz", tag="oz")
            ozs = oz[:].rearrange("p (h e) -> p h e", h=HPG)
            nc.gpsimd.memset(ozs[:, :, 0:1], 1.0)
            nc.gpsimd.memset(ozs[:, :, 1:2], 0.0)
            zer = consts.tile([P, SEQ], fp32, name="zer", tag="zer")
            nc.gpsimd.memset(zer[:], 0.0)

            # ---- projection: qkT[n, s] = sum_d wqk[d, n] * xT[d, s] + bqk[n]
            # Q^T stays packed two-heads-per-tile (it is the 128-row moving
            # operand of the scores matmul).  K^T goes into per-head tiles
            # zero-padded to 128 partitions, so the scores stationary is a
            # full (128,128) tile — same PE config as every other matmul
            # (the pad rows hit the odd head's Q rows and contribute 0).
            # K^T of head h sits at rows 64*(h%2)..+64 (aligned with that
            # head's Q^T rows in the packed q tile); the other 64 rows are 0.
            q_tiles = [qk_pool.tile([P, SEQ], cdt, name=f"q{nt}", tag=f"q{nt}") for nt in range(2)]
            kp_tiles = [qk_pool.tile([P, SEQ], cdt, name=f"kp{h}", tag=f"kp{h}") for h in range(HPG)]
            for h in range(HPG):
                zrow = 64 * (1 - h % 2)
                nc.vector.tensor_copy(kp_tiles[h][zrow:zrow + 64, :], zer[0:64, :])

            def proj_qk(nt):
                for sc in range(4):
                    o = sc * 512
                    ps = psP.tile([P, 512], fp32, name="psP", tag="psP")
                    for d in range(DT_TILES):
                        nc.tensor.matmul(
                            ps[:],
                            mm(wqk[d][:, nt * P:(nt + 1) * P]),
                            mm(xt[d][:, o:o + 512]),
                            start=(d == 0),
                            stop=(d == DT_TILES - 1),
                        )
                    if nt < 2:
                        nc.vector.tensor_scalar_add(
                            q_tiles[nt][:, o:o + 512], ps[:], bqk[:, nt:nt + 1]
                        )
                    else:
                        for hh in range(2):
                            h = (nt - 2) * 2 + hh
                            r = hh * 64
                            nc.vector.tensor_scalar_add(
                                kp_tiles[h][r:r + 64, o:o + 512],
                                ps[r:r + 64, :],
                                bqk[r:r + 64, nt:nt + 1],
                            )
                            nc.sync.dma_start(
                                kT_d[h * 64:(h + 1) * 64, o:o + 512],
                                kp_tiles[h][r:r + 64, o:o + 512],
                            )

            # ---- projection: v[s, n] = sum_d xT[d, s] * wv[d, n] + bv[n]
            # v4 tiles hold [V | 1] interleaved per head: cols h*65..h*65+63 = V,
            # col h*65+64 = ones (PV stationary with free denominator row).
            v4_tiles = []

            def proj_v(sts):
                for st in sts:
                    ps = psP.tile([P, 512], fp32, name="psP", tag="psP")
                    for d in range(DT_TILES):
                        nc.tensor.matmul(
                            ps[:, 0:NV],
                            mm(xt[d][:, st * P:(st + 1) * P]),
                            mm(wv[d][:]),
                            start=(d == 0),
                            stop=(d == DT_TILES - 1),
                        )
                    v4 = v4_pool.tile([P, HPG * HDP], cdt, name=f"v4_{st}", tag=f"v4_{st}")
                    v4r = v4[:].rearrange("p (h e) -> p h e", h=HPG)
                    v4s = v4r[:, :, 0:HD]
                    nc.vector.tensor_copy(v4r[:, :, HD:HDP], ozs)
                    nc.vector.tensor_add(
                        v4s,
                        ps[:, 0:NV].rearrange("p (h e) -> p h e", h=HPG),
                        bv[:].rearrange("p (h e) -> p h e", h=HPG),
                    )
                    nc.sync.dma_start(
                        v_d[st * P:(st + 1) * P, :].rearrange("p (h e) -> p h e", h=HPG),
                        v4s,
                    )
                    v4_tiles.append(v4)

            # ---- attention for one head, one q-chunk ----
            def attn(h, j, last=False):
                ktile = kp_tiles[h]
                qtile = q_tiles[h // 2]
                n_k = 8 * (j + 1)
                # diagonal k-tiles first: their serial exp chains overlap the
                # full-tile matmuls instead of sitting on the kernel tail.
                # The very last chunk instead ends on the SMALLEST diagonal
                # exp (m=7, 128 cols) so the closing serial chain is short.
                if last:
                    order = list(range(0, 8 * j)) + list(range(8 * j, n_k))
                else:
                    order = list(range(8 * j, n_k)) + list(range(0, 8 * j))
                halves = {0: [i for i in order if i - 8 * j < 4], 1: order}
                pso = psB.tile([HD + 2, QCH], fp32, name="pso", tag="pso")
                for i in order:
                    m = i - 8 * j  # >= 0 on the diagonal band
                    vs = max(P * m, 0)  # first causally-valid q col in chunk
                    pss = psA.tile([P, QCH], fp32, name="psA", tag="psA")
                    pt = pt_pool.tile([P, QCH], cdt, name="pt", tag="pt")
                    # per 512-half, only the causally valid segment [s0:s1):
                    # scores/PV stream just that range, so the masked-out
                    # left part of pt is never written NOR read.
                    segs = []
                    for half in range(2):
                        s0, s1 = max(half * 512, vs), (half + 1) * 512
                        if s0 < s1:
                            segs.append((half, s0, s1))
                    for half, s0, s1 in segs:
                        nc.tensor.matmul(
                            pss[:, s0:s1],
                            mm(ktile[:, i * P:(i + 1) * P]),
                            mm(qtile[:, j * QCH + s0:j * QCH + s1]),
                            start=True,
                            stop=True,
                        )
                    nc.scalar.activation(
                        pt[:, vs:QCH],
                        pss[:, vs:QCH],
                        mybir.ActivationFunctionType.Exp,
                        scale=0.125,
                    )
                    if m >= 0:
                        nc.vector.tensor_mul(pt[:, vs:vs + P], pt[:, vs:vs + P], tri[:])
                    for half, s0, s1 in segs:
                        nc.tensor.matmul(
                            pso[:, s0:s1],
                            mm(v4_tiles[i][:, h * HDP:(h + 1) * HDP]),
                            mm(pt[:, s0:s1]),
                            start=(i == halves[half][0]),
                            stop=(i == halves[half][-1]),
                        )
                osb = ot_pool.tile([HD + 1, QCH], fp32, name="osb", tag="osb")
                nc.vector.tensor_copy(osb[:], pso[0:HD + 1, :])
                nc.sync.dma_start(
                    outT_d[h * (HD + 1):(h + 1) * (HD + 1), j * QCH:(j + 1) * QCH],
                    osb[:],
                )

            # dovetail attention (ACT-heavy) with remaining projection
            # (PE-only): every attn(h,0) needs only v4[0:8] + q_tiles[h//2]
            # + kp[h], so the rest of the projection is spread between the
            # j=0 chunks; attn(h,1) additionally needs v4[8:16].
            proj_qk(0)
            proj_qk(2)
            proj_v(range(0, 8))
            attn(0, 0)
            proj_qk(1)
            attn(1, 0)
            proj_qk(3)
            attn(2, 0)
            proj_v(range(8, KT))
            attn(3, 0)
            attn(0, 1)
            attn(1, 1)
            attn(2, 1)
            attn(3, 1, last=True)

    nc.compile()
    return nc


def _get_program():
    if "nc" not in _CACHE:
        _CACHE["nc"] = _build_program()
    return _CACHE["nc"]


def _in_maps(x, W_qkv, b_qkv):
    cdt_np = np.float32 if CMODE != "bf16" else None
    if cdt_np is None:
        import ml_dtypes

        cdt_np = ml_dtypes.bfloat16
    tri = np.triu(np.ones((P, P), np.float32)).astype(cdt_np)
    maps = []
    for c in range(NCORES):
        b, hg = divmod(c, GROUPS)
        r0 = hg * NV
        wq = W_qkv[r0:r0 + NV]
        wk = W_qkv[D + r0:D + r0 + NV]
        wvs = W_qkv[2 * D + r0:2 * D + r0 + NV]
        maps.append({
            "xT": np.ascontiguousarray(x[b].T).astype(cdt_np),
            "wqk": np.ascontiguousarray(np.concatenate([wq, wk], 0).T).astype(cdt_np),
            "wv": np.ascontiguousarray(wvs.T).astype(cdt_np),
            "bqk": np.ascontiguousarray(
                np.concatenate([b_qkv[r0:r0 + NV], b_qkv[D + r0:D + r0 + NV]])[:, None]
            ).astype(np.float32),
            "bv": np.broadcast_to(
                b_qkv[2 * D + r0:2 * D + r0 + NV], (P, NV)
            ).astype(np.float32),
            "tri": tri,
        })
    return maps


def kernel(x, W_qkv, b_qkv, _trace=False):
    x = np.asarray(x, np.float32)
    W_qkv = np.asarray(W_qkv, np.float32)
    b_qkv = np.asarray(b_qkv, np.float32)

    from concourse.bass_utils import run_bass_kernel_spmd

    nc = _get_program()
    maps = _in_maps(x, W_qkv, b_qkv)
    kw = {}
    if _trace:
        import tempfile

        kw["tmpdir"] = tempfile.mkdtemp(prefix="trn_trace_")
        _CACHE["trace_dir"] = kw["tmpdir"]
    res = run_bass_kernel_spmd(nc, maps, list(range(NCORES)), trace=_trace, **kw)
    _CACHE["last_result"] = res

    out = np.empty((BS, SEQ, D), np.float32)
    K = np.empty((BS, H, SEQ, HD), np.float32)
    V = np.empty((BS, H, SEQ, HD), np.float32)
    for c in range(NCORES):
        b, hg = divmod(c, GROUPS)
        r = res.results[c]
        kTo = np.asarray(r["kT"], np.float32)
        vo = np.asarray(r["v"], np.float32)
        oT = np.asarray(r["outT"], np.float32).reshape(HPG, HD + 1, SEQ)
        for hh in range(HPG):
            h = hg * HPG + hh
            K[b, h] = kTo[hh * HD:(hh + 1) * HD, :].T
            V[b, h] = vo[:, hh * HD:(hh + 1) * HD]
            out[b][:, h * HD:(h + 1) * HD] = (oT[hh, :HD, :] / oT[hh, HD, :]).T
    return (out, (K, V))
